# revision 19
# baseline (speedup 1.0000x reference)
"""DigitCapsules routing kernel for 8 Trainium2 NeuronCores.

Math: in the reference, u_hat is an explicit broadcast of u_core over the
capsule axis i, so b stays constant along i in every routing iteration,
softmax over i is exactly uniform (1/K), and the whole 3-iteration routing
collapses (exactly, in floating point too) to:

    v[b, i, :] = squash((1/576) * sum_{r,k} x2[b, r, k] * W[b, r, k, :])

broadcast over i = 0..575, where x2 = x.reshape(B, 8, 576).transpose(0, 2, 1).

Sharding: batch dim B=32 across 8 cores, 4 batches per core (data parallel,
per the hint).

Per core, the contraction runs with W STATIONARY and x moving: per r-tile,
LDWEIGHTS loads W[128 routes, 128 (j,k') cols] (fp16, so the fast-weight-load
path applies) and the matmul streams only x's 8 columns — ~60 cycles per
tile instead of the 128-column-moving variant's 128, so the 20-tile
contraction is ~1.2us on the cold (1.2 GHz) PE instead of ~2.2us.  Each
batch's G[(j,k'), k] lands in its own [128, 8] PSUM bank; a per-batch
mask-multiply + free-axis reduce (both ~130 ns, hidden under the next
batch's matmuls) collapses the k-diagonal into one column of s1_all
[128, 4].  One fp32r matmul (stationary s1_all, moving the j-collector
mask) then produces T[4, 16] = sum over routes for all four batches; squash
runs once over [4, *]; one fp16 select-matmul spreads v to all 128
partitions; ScalarE evicts PSUM; ONE [128 x 1152B] DMA stores the
i-broadcast output using all 16 DMA engines.

Performance notes:
 - The measured exec window includes ~8.2us of fixed NEFF machinery (runtime
   preamble const-memsets anchor the window start; NRT injects ~253
   per-semaphore reset instructions before the final branch).  Only the
   body is optimizable.
 - The host packs wx = [W | x2] rows partition-major, pair-merged (2.7 KB
   per-partition DMA runs) and pre-converted to fp16: halves HBM traffic
   and enables FWL (PSUM accumulation stays fp32; measured end-to-end
   relative error ~4e-4 against the fp32 reference).
 - The k-diagonal / j-collector masks are host-packed constants (their
   (j,k')-major layout is not affine-expressible for on-device
   affine_select in this orientation); they ride in one tiny early DMA.
 - Bacc (not raw Bass): its compile() splits sync waits into event
   semaphores (TRN2 allows one wait per instruction).
 - tensor_tensor_reduce (custom DVE op) hard-crashes the exec unit on this
   runtime - avoid.
"""

import numpy as np

import concourse.bacc as bacc
import concourse.mybir as mybir
import concourse.tile as tile
from concourse.bass_utils import run_bass_kernel_spmd

N_CORES = 8
B, C, H, W_ = 32, 8, 24, 24
R = H * W_          # 576 routes
RP = 640            # padded routes (5 tiles of 128)
KJ = 128            # fused (j=16, k'=8) axis
D = 16
NB = B // N_CORES   # 4 batches per core
NTILE = RP // 128   # 5
WX = KJ + C         # 136 = W row + packed x2 row
FREE = NTILE * WX   # 680 fp16 values per partition
RNORM = 1.0 / float(R)
RNORM2 = RNORM * RNORM

_cached_nc = None
_last_in_maps = None


def _build():
    nc = bacc.Bacc(trn_type="TRN2")
    f32 = mybir.dt.float32
    f32r = mybir.dt.float32r
    f16 = mybir.dt.float16

    wx_h = nc.dram_tensor("wx", [NB // 2, 128, 2 * FREE], f16,
                          kind="ExternalInput")
    # maskT[(j,k'), k] = (k' == k);  maskJ[(j,k'), j2] = (j == j2)
    mkt_h = nc.dram_tensor("mkt", [128, 8], f32, kind="ExternalInput")
    mkj_h = nc.dram_tensor("mkj", [128, 16], f32r, kind="ExternalInput")
    out_h = nc.dram_tensor("out", [NB, R, D], f32, kind="ExternalOutput")

    with tile.TileContext(nc) as tc:
        with (
            tc.tile_pool(name="consts", bufs=1) as consts,
            tc.tile_pool(name="wp", bufs=NB) as wp,
            tc.tile_pool(name="gps", bufs=NB, space="PSUM") as gps,
            tc.tile_pool(name="tps", bufs=1, space="PSUM") as tps,
            tc.tile_pool(name="vps", bufs=1, space="PSUM") as vps,
            tc.tile_pool(name="sm", bufs=16) as sm,
        ):
            mkt_t = consts.tile([128, 8], f32)
            nc.scalar.dma_start(mkt_t[:], mkt_h[:, :])
            mkj_t = consts.tile([128, 16], f32r)
            nc.scalar.dma_start(mkj_t[:], mkj_h[:, :])
            mask_t = mkt_t[:]
            mask_j = mkj_t[:]

            eps_t = consts.tile([NB, 1], f32)
            nc.vector.memset(eps_t[:], 1e-8)
            # sel4[n, p] = (p // 32 == n): spreads v across all 128
            # partitions so the output DMA engages all 16 DMA engines
            # (engine = source partition mod 16).  fp16 so the broadcast
            # matmul gets FWL and 1-cycle/column streaming.
            sel_raw = consts.tile([NB, 128], f32)
            nc.gpsimd.memset(sel_raw[:], 1.0)
            nc.gpsimd.affine_select(
                out=sel_raw[:], in_=sel_raw[:],
                compare_op=mybir.AluOpType.is_ge, fill=0.0,
                base=0, pattern=[[1, 128]], channel_multiplier=-32,
            )
            nc.gpsimd.affine_select(
                out=sel_raw[:], in_=sel_raw[:],
                compare_op=mybir.AluOpType.is_ge, fill=0.0,
                base=31, pattern=[[-1, 128]], channel_multiplier=32,
            )
            sel4 = consts.tile([NB, 128], f16)
            nc.vector.tensor_copy(sel4[:], sel_raw[:])

            # Per-batch contraction G[n][(j,k'), k] = sum_r W[n,r,(j,k')] *
            # x2[n,r,k] (W stationary, x moving), then the k-diagonal
            # partials s1_all[(j,k'), n] = sum_k G[n][(j,k'), k]*(k'==k).
            s1_all = sm.tile([128, NB], f32r)
            for pr in range(NB // 2):
                wx_t = wp.tile([128, 2 * FREE], f16)
                # One DMA per batch pair: 2.7 KB per-partition runs halve
                # the descriptor count; a single dma_start stripes its
                # partition rows over all 16 DMA engines.
                eng = nc.sync if pr % 2 == 0 else nc.scalar
                eng.dma_start(wx_t[:], wx_h[pr])
                wx_v = wx_t[:].rearrange("p (d f) -> p d f", f=WX)
                for nl in range(2):
                    n = 2 * pr + nl
                    g_b = gps.tile([128, C], f32, tag="g_b")
                    for d in range(NTILE):
                        nc.tensor.matmul(
                            g_b[:],
                            wx_v[:, nl * NTILE + d, :KJ],
                            wx_v[:, nl * NTILE + d, KJ:WX],
                            start=(d == 0), stop=(d == NTILE - 1),
                        )
                    pm = sm.tile([128, C], f32, tag="pm")
                    nc.vector.tensor_mul(pm[:], g_b[:], mask_t)
                    # f32r out has the same bits as f32 — only tagged so
                    # the collect matmul runs single-pass, not LOW/HIGH.
                    with nc.allow_low_precision("f32r == f32 bitwise"):
                        nc.vector.reduce_sum(
                            s1_all[:, n:n + 1], pm[:],
                            axis=mybir.AxisListType.X,
                        )

            # T[n, j] = sum_{(j',k')} s1_all[(j',k'), n] * (j' == j): one
            # fp32r matmul collects the route sums for all four batches.
            t_all = tps.tile([NB, D], f32)
            nc.tensor.matmul(
                t_all[:], s1_all[:], mask_j, start=True, stop=True,
            )

            # Batched squash over all 4 batches:
            #   normT = sum_j T^2;  norm = normT/576^2
            #   v = T * (norm/576) / ((1+norm) * sqrt(norm + 1e-8))
            # (square on DVE: scalar.square would evict Sqrt's ACT table)
            t_sb = sm.tile([NB, D], f32)
            nc.vector.tensor_copy(t_sb[:], t_all[:])
            sq = sm.tile([NB, D], f32)
            nc.vector.tensor_mul(sq[:], t_sb[:], t_sb[:])
            norm_t = sm.tile([NB, 1], f32)
            nc.vector.reduce_sum(norm_t[:], sq[:], axis=mybir.AxisListType.X)
            q = sm.tile([NB, 1], f32)
            nc.scalar.activation(
                q[:], norm_t[:], mybir.ActivationFunctionType.Sqrt,
                bias=eps_t[:], scale=RNORM2,
            )
            a1 = sm.tile([NB, 1], f32)
            nc.vector.tensor_scalar(
                out=a1[:], in0=norm_t[:], scalar1=RNORM2, scalar2=1.0,
                op0=mybir.AluOpType.mult, op1=mybir.AluOpType.add,
            )
            den = sm.tile([NB, 1], f32)
            nc.vector.tensor_mul(den[:], a1[:], q[:])
            rec = sm.tile([NB, 1], f32)
            nc.vector.reciprocal(rec[:], den[:])
            c1 = sm.tile([NB, 1], f32)
            nc.vector.tensor_scalar_mul(c1[:], norm_t[:], RNORM2 * RNORM)
            v1 = sm.tile([NB, D], f16)
            nc.vector.tensor_scalar(
                out=v1[:], in0=t_all[:], scalar1=c1[:], scalar2=rec[:],
                op0=mybir.AluOpType.mult, op1=mybir.AluOpType.mult,
            )

            # Spread v across all 128 partitions (partition p gets batch
            # p//32's v, 18 copies) and store with ONE 147 KB DMA that
            # engages all 16 DMA engines.  The PSUM evict runs on ScalarE
            # (closer to PSUM: 383 ns vs DVE's 425 for [*, 288]).
            vb_ps = vps.tile([128, 18 * D], f32)
            nc.tensor.matmul(
                vb_ps[:], sel4[:],
                v1[:].unsqueeze(1).broadcast_to([NB, 18, D]),
                start=True, stop=True)
            vb = sm.tile([128, 18 * D], f32)
            nc.scalar.copy(vb[:], vb_ps[:])
            dst = out_h[:, :, :].flatten().rearrange(
                "(p c) -> p c", c=18 * D)
            nc.sync.dma_start(dst, vb[:])

    nc.finalize()
    return nc


def _make_masks():
    # partition p = j*8 + k' (j-major W column packing)
    jj = np.arange(128) // 8
    kk = np.arange(128) % 8
    mkt = np.zeros((128, 8), np.float32)
    mkt[np.arange(128), kk] = 1.0                   # maskT: k' == k
    mkj = np.zeros((128, 16), np.float32)
    mkj[np.arange(128), jj] = 1.0                   # maskJ: j' == j2
    return mkt, mkj


def kernel(x, route_weights):
    global _cached_nc, _last_in_maps
    if _cached_nc is None:
        _cached_nc = _build()
    nc = _cached_nc

    x = np.ascontiguousarray(np.asarray(x), dtype=np.float32)
    w = np.ascontiguousarray(np.asarray(route_weights), dtype=np.float32)
    x2 = x.reshape(B, C, R).transpose(0, 2, 1)          # [B, R, 8]
    # j-major column packing: wf[b, r, j*8+k] = W[b, r, k, j]
    wf = w.reshape(B, R, C, D).transpose(0, 1, 3, 2).reshape(B, R, KJ)
    wx = np.zeros((B, RP, WX), np.float32)
    wx[:, :R, :KJ] = wf
    wx[:, :R, KJ:] = x2
    # partition-major tiling, fp16, pair-merged: [B/2, 128, 2*NTILE*WX]
    wxt = (wx.reshape(B, NTILE, 128, WX).transpose(0, 2, 1, 3)
           .reshape(B // 2, 2, 128, FREE).transpose(0, 2, 1, 3)
           .reshape(B // 2, 128, 2 * FREE)).astype(np.float16)

    mkt, mkj = _make_masks()
    npair = NB // 2
    in_maps = [
        {"wx": np.ascontiguousarray(wxt[c * npair:(c + 1) * npair]),
         "mkt": mkt, "mkj": mkj}
        for c in range(N_CORES)
    ]
    _last_in_maps = in_maps

    res = run_bass_kernel_spmd(nc, in_maps, core_ids=list(range(N_CORES)))
    return np.concatenate([r["out"] for r in res.results], axis=0)


# revision 22
# speedup vs baseline: 1.0426x; 1.0426x over previous
"""DigitCapsules routing kernel for 8 Trainium2 NeuronCores.

Math: in the reference, u_hat is an explicit broadcast of u_core over the
capsule axis i, so b stays constant along i in every routing iteration,
softmax over i is exactly uniform (1/K), and the whole 3-iteration routing
collapses (exactly, in floating point too) to:

    v[b, i, :] = squash((1/576) * sum_{r,k} x2[b, r, k] * W[b, r, k, :])

broadcast over i = 0..575, where x2 = x.reshape(B, 8, 576).transpose(0, 2, 1).

Sharding: batch dim B=32 across 8 cores, 4 batches per core (data parallel,
per the hint).

Per core, the contraction runs with W STATIONARY and x moving: per r-tile,
LDWEIGHTS loads W[128 routes, 128 (j,k') cols] (fp16, so the fast-weight-load
path applies) and the matmul streams only x's 8 columns — ~60 cycles per
tile instead of the 128-column-moving variant's 128, so the 20-tile
contraction is ~1.2us on the cold (1.2 GHz) PE instead of ~2.2us.  Each
batch's G[(j,k'), k] lands in its own [128, 8] PSUM bank; a per-batch
mask-multiply + free-axis reduce (both ~130 ns, hidden under the next
batch's matmuls) collapses the k-diagonal into one column of s1_all
[128, 4].  One fp32r matmul (stationary s1_all, moving the j-collector
mask) then produces T[4, 16] = sum over routes for all four batches; squash
runs once over [4, *]; one fp16 select-matmul spreads v to all 128
partitions; ScalarE evicts PSUM; ONE [128 x 1152B] DMA stores the
i-broadcast output using all 16 DMA engines.

Performance notes:
 - The measured exec window includes ~8.2us of fixed NEFF machinery (runtime
   preamble const-memsets anchor the window start; NRT injects ~253
   per-semaphore reset instructions before the final branch).  Only the
   body is optimizable.
 - The host packs wx = [W | x2] rows partition-major, pair-merged (2.7 KB
   per-partition DMA runs) and pre-converted to fp16: halves HBM traffic
   and enables FWL (PSUM accumulation stays fp32; measured end-to-end
   relative error ~4e-4 against the fp32 reference).
 - The k-diagonal / j-collector masks are host-packed constants (their
   (j,k')-major layout is not affine-expressible for on-device
   affine_select in this orientation); they ride in one tiny early DMA.
 - Bacc (not raw Bass): its compile() splits sync waits into event
   semaphores (TRN2 allows one wait per instruction).
 - tensor_tensor_reduce (custom DVE op) hard-crashes the exec unit on this
   runtime - avoid.
"""

import numpy as np

import concourse.bacc as bacc
import concourse.mybir as mybir
import concourse.tile as tile
from concourse.bass_utils import run_bass_kernel_spmd

N_CORES = 8
B, C, H, W_ = 32, 8, 24, 24
R = H * W_          # 576 routes
RP = 640            # padded routes (5 tiles of 128)
KJ = 128            # fused (j=16, k'=8) axis
D = 16
NB = B // N_CORES   # 4 batches per core
NTILE = RP // 128   # 5
WX = KJ + C         # 136 = W row + packed x2 row
FREE = NTILE * WX   # 680 fp16 values per partition
RNORM = 1.0 / float(R)
RNORM2 = RNORM * RNORM

_cached_nc = None
_last_in_maps = None


def _build():
    nc = bacc.Bacc(trn_type="TRN2")
    f32 = mybir.dt.float32
    f32r = mybir.dt.float32r
    f16 = mybir.dt.float16

    wx_h = nc.dram_tensor("wx", [NB // 2, 128, 2 * FREE], f16,
                          kind="ExternalInput")
    # maskT[(j,k'), k] = (k' == k);  maskJ[(j,k'), j2] = (j == j2)
    mkt_h = nc.dram_tensor("mkt", [128, 8], f32, kind="ExternalInput")
    mkj_h = nc.dram_tensor("mkj", [128, 16], f32r, kind="ExternalInput")
    out_h = nc.dram_tensor("out", [NB, R, D], f32, kind="ExternalOutput")

    with tile.TileContext(nc) as tc:
        with (
            tc.tile_pool(name="consts", bufs=1) as consts,
            tc.tile_pool(name="wp", bufs=NB) as wp,
            tc.tile_pool(name="gps", bufs=NB, space="PSUM") as gps,
            tc.tile_pool(name="tps", bufs=1, space="PSUM") as tps,
            tc.tile_pool(name="vps", bufs=1, space="PSUM") as vps,
            tc.tile_pool(name="sm", bufs=16) as sm,
        ):
            # Input DMAs go FIRST on both HWDGE rings — the rings drain
            # FIFO per issuing engine, so anything queued ahead of the
            # pair-1 transfer delays the whole contraction.
            wx_tiles = []
            for pr in range(NB // 2):
                wx_t = wp.tile([128, 2 * FREE], f16)
                # One DMA per batch pair: 2.7 KB per-partition runs halve
                # the descriptor count; a single dma_start stripes its
                # partition rows over all 16 DMA engines.
                eng = nc.sync if pr % 2 == 0 else nc.scalar
                eng.dma_start(wx_t[:], wx_h[pr])
                wx_tiles.append(wx_t)

            mkt_t = consts.tile([128, 8], f32)
            nc.sync.dma_start(mkt_t[:], mkt_h[:, :])
            mkj_t = consts.tile([128, 16], f32r)
            nc.scalar.dma_start(mkj_t[:], mkj_h[:, :])
            mask_t = mkt_t[:]
            mask_j = mkj_t[:]

            eps_t = consts.tile([NB, 1], f32)
            nc.vector.memset(eps_t[:], 1e-8)
            # sel4[n, p] = (p // 32 == n): spreads v across all 128
            # partitions so the output DMA engages all 16 DMA engines
            # (engine = source partition mod 16).  fp16 so the broadcast
            # matmul gets FWL and 1-cycle/column streaming.
            sel_raw = consts.tile([NB, 128], f32)
            nc.gpsimd.memset(sel_raw[:], 1.0)
            nc.gpsimd.affine_select(
                out=sel_raw[:], in_=sel_raw[:],
                compare_op=mybir.AluOpType.is_ge, fill=0.0,
                base=0, pattern=[[1, 128]], channel_multiplier=-32,
            )
            nc.gpsimd.affine_select(
                out=sel_raw[:], in_=sel_raw[:],
                compare_op=mybir.AluOpType.is_ge, fill=0.0,
                base=31, pattern=[[-1, 128]], channel_multiplier=32,
            )
            sel4 = consts.tile([NB, 128], f16)
            nc.vector.tensor_copy(sel4[:], sel_raw[:])

            # Per-batch contraction G[n][(j,k'), k] = sum_r W[n,r,(j,k')] *
            # x2[n,r,k] (W stationary, x moving), then the k-diagonal
            # partials s1_all[(j,k'), n] = sum_k G[n][(j,k'), k]*(k'==k).
            s1_all = sm.tile([128, NB], f32r)
            for pr in range(NB // 2):
                wx_v = wx_tiles[pr][:].rearrange("p (d f) -> p d f", f=WX)
                for nl in range(2):
                    n = 2 * pr + nl
                    g_b = gps.tile([128, C], f32, tag="g_b")
                    for d in range(NTILE):
                        nc.tensor.matmul(
                            g_b[:],
                            wx_v[:, nl * NTILE + d, :KJ],
                            wx_v[:, nl * NTILE + d, KJ:WX],
                            start=(d == 0), stop=(d == NTILE - 1),
                        )
                    pm = sm.tile([128, C], f32, tag="pm")
                    nc.vector.tensor_mul(pm[:], g_b[:], mask_t)
                    # f32r out has the same bits as f32 — only tagged so
                    # the collect matmul runs single-pass, not LOW/HIGH.
                    with nc.allow_low_precision("f32r == f32 bitwise"):
                        nc.vector.reduce_sum(
                            s1_all[:, n:n + 1], pm[:],
                            axis=mybir.AxisListType.X,
                        )

            # T[n, j] = sum_{(j',k')} s1_all[(j',k'), n] * (j' == j): one
            # fp32r matmul collects the route sums for all four batches.
            t_all = tps.tile([NB, D], f32)
            nc.tensor.matmul(
                t_all[:], s1_all[:], mask_j, start=True, stop=True,
            )

            # Batched squash over all 4 batches:
            #   normT = sum_j T^2;  norm = normT/576^2
            #   v = T * (norm/576) / ((1+norm) * sqrt(norm + 1e-8))
            # (square on DVE: scalar.square would evict Sqrt's ACT table)
            t_sb = sm.tile([NB, D], f32)
            nc.vector.tensor_copy(t_sb[:], t_all[:])
            sq = sm.tile([NB, D], f32)
            nc.vector.tensor_mul(sq[:], t_sb[:], t_sb[:])
            norm_t = sm.tile([NB, 1], f32)
            nc.vector.reduce_sum(norm_t[:], sq[:], axis=mybir.AxisListType.X)
            q = sm.tile([NB, 1], f32)
            nc.scalar.activation(
                q[:], norm_t[:], mybir.ActivationFunctionType.Sqrt,
                bias=eps_t[:], scale=RNORM2,
            )
            a1 = sm.tile([NB, 1], f32)
            nc.vector.tensor_scalar(
                out=a1[:], in0=norm_t[:], scalar1=RNORM2, scalar2=1.0,
                op0=mybir.AluOpType.mult, op1=mybir.AluOpType.add,
            )
            den = sm.tile([NB, 1], f32)
            nc.vector.tensor_mul(den[:], a1[:], q[:])
            rec = sm.tile([NB, 1], f32)
            nc.vector.reciprocal(rec[:], den[:])
            c1 = sm.tile([NB, 1], f32)
            nc.vector.tensor_scalar_mul(c1[:], norm_t[:], RNORM2 * RNORM)
            v1 = sm.tile([NB, D], f16)
            nc.vector.tensor_scalar(
                out=v1[:], in0=t_all[:], scalar1=c1[:], scalar2=rec[:],
                op0=mybir.AluOpType.mult, op1=mybir.AluOpType.mult,
            )

            # Spread v across all 128 partitions (partition p gets batch
            # p//32's v, 18 copies) and store with ONE 147 KB DMA that
            # engages all 16 DMA engines.  The PSUM evict runs on ScalarE
            # (closer to PSUM: 383 ns vs DVE's 425 for [*, 288]).
            vb_ps = vps.tile([128, 18 * D], f32)
            nc.tensor.matmul(
                vb_ps[:], sel4[:],
                v1[:].unsqueeze(1).broadcast_to([NB, 18, D]),
                start=True, stop=True)
            vb = sm.tile([128, 18 * D], f32)
            nc.vector.tensor_copy(vb[:], vb_ps[:])
            dst = out_h[:, :, :].flatten().rearrange(
                "(p c) -> p c", c=18 * D)
            nc.sync.dma_start(dst, vb[:])

    nc.finalize()
    return nc


def _make_masks():
    # partition p = j*8 + k' (j-major W column packing)
    jj = np.arange(128) // 8
    kk = np.arange(128) % 8
    mkt = np.zeros((128, 8), np.float32)
    mkt[np.arange(128), kk] = 1.0                   # maskT: k' == k
    mkj = np.zeros((128, 16), np.float32)
    mkj[np.arange(128), jj] = 1.0                   # maskJ: j' == j2
    return mkt, mkj


def kernel(x, route_weights):
    global _cached_nc, _last_in_maps
    if _cached_nc is None:
        _cached_nc = _build()
    nc = _cached_nc

    x = np.ascontiguousarray(np.asarray(x), dtype=np.float32)
    w = np.ascontiguousarray(np.asarray(route_weights), dtype=np.float32)
    x2 = x.reshape(B, C, R).transpose(0, 2, 1)          # [B, R, 8]
    # j-major column packing: wf[b, r, j*8+k] = W[b, r, k, j]
    wf = w.reshape(B, R, C, D).transpose(0, 1, 3, 2).reshape(B, R, KJ)
    wx = np.zeros((B, RP, WX), np.float32)
    wx[:, :R, :KJ] = wf
    wx[:, :R, KJ:] = x2
    # partition-major tiling, fp16, pair-merged: [B/2, 128, 2*NTILE*WX]
    wxt = (wx.reshape(B, NTILE, 128, WX).transpose(0, 2, 1, 3)
           .reshape(B // 2, 2, 128, FREE).transpose(0, 2, 1, 3)
           .reshape(B // 2, 128, 2 * FREE)).astype(np.float16)

    mkt, mkj = _make_masks()
    npair = NB // 2
    in_maps = [
        {"wx": np.ascontiguousarray(wxt[c * npair:(c + 1) * npair]),
         "mkt": mkt, "mkj": mkj}
        for c in range(N_CORES)
    ]
    _last_in_maps = in_maps

    res = run_bass_kernel_spmd(nc, in_maps, core_ids=list(range(N_CORES)))
    return np.concatenate([r["out"] for r in res.results], axis=0)
